# revision 3
# baseline (speedup 1.0000x reference)
"""Trainium2 Bass kernel for the BinaryMechanismSSM problem.

Full inputs in, full outputs out. Batch (128) sharded 8 ways (16 rows/core).

Per core:
  Phase 1: projections bx{0,1} = x @ B{0,1}^T + b (fp16 matmuls, fp32 psum),
           g = sigmoid(x @ G^T + b). Staged to DRAM planes in fp16, already
           laid out group-interleaved as (t, m, c, b) / (t, c, b) columns so
           phase 2 reads one contiguous slab per 16-step group.
  Phase 2: T sequential steps. State held fp16 as s[h][p, 16*cl+b] =
           state[b, 128*(2h+cl)+p] (two [128, 32] half tiles). Per step:
           one fp16 identity matmul injects bx into PSUM (also sets
           has_written for the whole bank), 32 fp16 A-matmuls accumulate,
           then per half: ACT tanh -> DVE (alpha*g)*tanh -> pair-add ->
           + (1-g)*s_prev, emitting the new state half (fp16) which gates
           the next step's matmuls for the corresponding contraction
           chunks. Matmul emission is grouped so each psum half completes
           as early as its newest state half allows.
"""
import numpy as np

B_FULL = 128
T_FULL = 1024
I_DIM = 256
S_DIM = 512
N_CORES = 8
B_LOC = B_FULL // N_CORES  # 16

_cache = {}


def _build(alpha: float, z: int, T: int):
    import concourse.bass as bass
    from concourse import bacc
    import concourse.mybir as mybir
    from concourse.tile import TileContext

    dt = mybir.dt
    AF = mybir.ActivationFunctionType
    ALU = mybir.AluOpType

    TOK = T * B_LOC          # tokens per core
    NTT = TOK // 512         # phase-1 token tiles (32 timesteps each)
    NG = T // 16             # phase-2 step groups
    NMAT = 3 if z != 0 else 2
    NREC = 2 if z != 0 else 1
    W = NREC * 64            # psum width per step
    a_coef = [1.0 - alpha, alpha] if z != 0 else [1.0]

    nc = bacc.Bacc("TRN2", target_bir_lowering=False, debug=False,
                   num_devices=N_CORES)

    xT_d = nc.declare_dram_parameter("xT", [2, 128, TOK], dt.float16, isOutput=False)
    pw_d = nc.declare_dram_parameter("pw", [128, NMAT * 2 * 4 * 128], dt.float16, isOutput=False)
    # bias cols: (mat, c) for NMAT mats, then 4 cols of negated gate bias (unused now)
    bias_d = nc.declare_dram_parameter("bias", [128, 4 * NMAT], dt.float32, isOutput=False)
    aw_d = nc.declare_dram_parameter("aw", [128, NREC * 16 * 128], dt.float16, isOutput=False)
    s0_d = nc.declare_dram_parameter("s0T", [128, 64], dt.float16, isOutput=False)
    iden_d = nc.declare_dram_parameter("iden", [128, 128], dt.float16, isOutput=False)
    stg_d = nc.declare_dram_parameter("stg", [T, 128, 64], dt.float16, isOutput=True)

    with TileContext(nc) as tc:
      with tc.tile_pool(name="dram", bufs=1, space="DRAM") as dpool:
        # group-interleaved staging planes
        bxp = dpool.tile([128, T * NREC * 64], dt.float16, tag="bxp", name="bxp")
        hgp = dpool.tile([128, T * 64], dt.float16, tag="hgp", name="hgp")
        gmp = dpool.tile([128, T * 64], dt.float16, tag="gmp", name="gmp")
        bxp_v = bxp[:].rearrange("q (t m c b) -> q m c t b", t=T, m=NREC, c=4, b=16)
        hgp_v = hgp[:].rearrange("q (t c b) -> q c t b", t=T, c=4, b=16)
        gmp_v = gmp[:].rearrange("q (t c b) -> q c t b", t=T, c=4, b=16)

        # ---------------- Phase 1: projections ----------------
        with (
            tc.tile_pool(name="p1w", bufs=1) as p1w,
            tc.tile_pool(name="p1x", bufs=3) as p1x,
            tc.tile_pool(name="p1o", bufs=8) as p1o,
            tc.tile_pool(name="p1ps", bufs=6, space="PSUM") as p1ps,
        ):
            pw = p1w.tile([128, NMAT * 2 * 4 * 128], dt.float16)
            nc.sync.dma_start(pw[:], pw_d[:])
            bias = p1w.tile([128, 4 * NMAT], dt.float32)
            nc.sync.dma_start(bias[:], bias_d[:])

            for tt in range(NTT):
                t0 = tt * 32
                xt = p1x.tile([128, 2 * 512], dt.float16, tag="xt")
                for i in range(2):
                    nc.sync.dma_start(xt[:, i * 512:(i + 1) * 512],
                                      xT_d[i, :, tt * 512:(tt + 1) * 512])
                for mat in range(NMAT):
                    for c in range(4):
                        ps = p1ps.tile([128, 512], dt.float32, tag="pps")
                        for i in range(2):
                            blk = ((mat * 2 + i) * 4 + c) * 128
                            nc.tensor.matmul(
                                ps[:], pw[:, blk:blk + 128],
                                xt[:, i * 512:(i + 1) * 512],
                                start=(i == 0), stop=(i == 1))
                        bj = bias[:, mat * 4 + c:mat * 4 + c + 1]
                        if mat < NREC:
                            # bias add + fp16 cast on DVE
                            o = p1o.tile([128, 512], dt.float16, tag="po")
                            nc.vector.tensor_scalar(
                                o[:], ps[:], bj, None, ALU.add)
                            nc.sync.dma_start(
                                bxp_v[:, mat, c, t0:t0 + 32, :], o[:])
                        else:
                            sg = p1o.tile([128, 512], dt.float16, tag="psg")
                            nc.scalar.activation(sg[:], ps[:], AF.Sigmoid,
                                                 bias=bj, scale=1.0)
                            nc.sync.dma_start(
                                hgp_v[:, c, t0:t0 + 32, :], sg[:])
                            gm = p1o.tile([128, 512], dt.float16, tag="pgm")
                            nc.vector.tensor_scalar(
                                gm[:], sg[:], -1.0, 1.0, ALU.mult, ALU.add)
                            nc.sync.dma_start(
                                gmp_v[:, c, t0:t0 + 32, :], gm[:])

        # ---------------- Phase 2: recurrence ----------------
        with (
            tc.tile_pool(name="p2w", bufs=1) as p2w,
            tc.tile_pool(name="p2g", bufs=2) as p2g,
            tc.tile_pool(name="p2s", bufs=3) as p2s,
            tc.tile_pool(name="p2c", bufs=6) as p2c,
            tc.tile_pool(name="p2ps", bufs=4, space="PSUM") as p2ps,
        ):
            aw = p2w.tile([128, NREC * 16 * 128], dt.float16)
            nc.sync.dma_start(aw[:], aw_d[:])
            iden = p2w.tile([128, 128], dt.float16)
            nc.sync.dma_start(iden[:], iden_d[:])

            sth = []
            for h in range(2):
                s_init = p2s.tile([128, 32], dt.float16, tag=f"sth{h}")
                nc.sync.dma_start(s_init[:], s0_d[:, h * 32:(h + 1) * 32])
                sth.append(s_init)

            def ablk(m, c, k):
                return ((m * 4 + c) * 4 + k) * 128

            for g in range(NG):
                bxg = p2g.tile([128, 16 * NREC * 64], dt.float16, tag="bxg")
                nc.sync.dma_start(bxg[:], bxp[:, g * 16 * W:(g + 1) * 16 * W])
                hgg = p2g.tile([128, 16 * 64], dt.float16, tag="hgg")
                nc.sync.dma_start(hgg[:], hgp[:, g * 1024:(g + 1) * 1024])
                gmg = p2g.tile([128, 16 * 64], dt.float16, tag="gmg")
                nc.sync.dma_start(gmg[:], gmp[:, g * 1024:(g + 1) * 1024])

                for tt in range(16):
                    t = g * 16 + tt
                    pscat = p2ps.tile([128, W], dt.float32, tag="pscat")
                    # bx injection: one fp16 identity matmul, starts the bank
                    nc.tensor.matmul(
                        pscat[:], iden[:], bxg[:, tt * W:(tt + 1) * W],
                        start=True, stop=False)
                    # m2 = (1 - g_t) * s_t  (fp16, off the serial path)
                    m2 = p2c.tile([128, 64], dt.float16, tag="m2")
                    for h in range(2):
                        nc.vector.tensor_tensor(
                            m2[:, h * 32:(h + 1) * 32], sth[h][:],
                            gmg[:, tt * 64 + h * 32:tt * 64 + h * 32 + 32],
                            ALU.mult)
                    # A matmuls: regions (m, c); contraction chunk k reads
                    # sth[k//2].  Group so each psum half completes early.
                    for creg, kk in (((2, 3), (0, 1)), ((0, 1), (0, 1)),
                                     ((0, 1), (2, 3)), ((2, 3), (2, 3))):
                        for c in creg:
                            for m in range(NREC):
                                for k in kk:
                                    nc.tensor.matmul(
                                        pscat[:, m * 64 + c * 16:
                                              m * 64 + c * 16 + 16],
                                        aw[:, ablk(m, c, k):ablk(m, c, k) + 128],
                                        sth[k // 2][:, (k % 2) * 16:
                                                    (k % 2) * 16 + 16],
                                        start=False, stop=(k == 3))
                    # tails per half h (c pair {2h, 2h+1})
                    ps_v = pscat[:].rearrange("p (m h x) -> p h m x",
                                              m=NREC, h=2, x=32)
                    new_sth = [None, None]
                    for h in (0, 1):
                        ft = p2c.tile([128, NREC * 32], dt.float16, tag=f"ft{h}")
                        nc.scalar.activation(ft[:], ps_v[:, h], AF.Tanh)
                        hgs = hgg[:, tt * 64 + h * 32:tt * 64 + h * 32 + 32]
                        mc = p2c.tile([128, NREC * 32], dt.float16, tag=f"mc{h}")
                        for m in range(NREC):
                            nc.vector.scalar_tensor_tensor(
                                mc[:, m * 32:(m + 1) * 32],
                                ft[:, m * 32:(m + 1) * 32], a_coef[m],
                                hgs, ALU.mult, ALU.mult)
                        s_new = p2s.tile([128, 32], dt.float16, tag=f"sth{h}")
                        if NREC == 2:
                            u = p2c.tile([128, 32], dt.float16, tag=f"u{h}")
                            nc.vector.tensor_tensor(
                                u[:], mc[:, 0:32], mc[:, 32:64], ALU.add)
                        else:
                            u = mc
                        nc.vector.tensor_tensor(
                            s_new[:], u[:], m2[:, h * 32:(h + 1) * 32], ALU.add)
                        nc.sync.dma_start(stg_d[t][:, h * 32:(h + 1) * 32],
                                          s_new[:])
                        new_sth[h] = s_new
                    sth = new_sth

    nc.compile()
    return nc


def _pack_lhsT_blocks(Wm, kdim, mdim, dtype):
    """Wm: [mdim*128, kdim*128]; returns [128, kdim*mdim*128] with block
    (k, j) at cols (k*mdim+j)*128 equal to Wm[j-chunk, k-chunk].T."""
    out = np.zeros((128, kdim * mdim * 128), dtype=dtype)
    for k in range(kdim):
        for j in range(mdim):
            blk = Wm[j * 128:(j + 1) * 128, k * 128:(k + 1) * 128].T
            out[:, (k * mdim + j) * 128:(k * mdim + j + 1) * 128] = blk
    return np.ascontiguousarray(out)


def kernel(x_seq, s0, A0_w, B0_w, B0_b, A1_w, B1_w, B1_b, gate_w, gate_b,
           alpha, z, _T=None, _trace=False):
    from concourse.bass_utils import run_bass_kernel_spmd

    T = int(_T or T_FULL)
    alpha_f = float(np.asarray(alpha))
    z_i = int(np.asarray(z))

    key = (alpha_f, z_i, T)
    if key not in _cache:
        _cache[key] = _build(alpha_f, z_i, T)
    nc = _cache[key]

    NMAT = 3 if z_i != 0 else 2
    NREC = 2 if z_i != 0 else 1

    x_seq = np.asarray(x_seq, dtype=np.float32)
    s0 = np.asarray(s0, dtype=np.float32)

    # ---- shared (replicated) weight packing ----
    mats = ([np.asarray(B0_w), np.asarray(B1_w), np.asarray(gate_w)]
            if z_i != 0 else [np.asarray(B0_w), np.asarray(gate_w)])
    biases = ([np.asarray(B0_b), np.asarray(B1_b), np.asarray(gate_b)]
              if z_i != 0 else [np.asarray(B0_b), np.asarray(gate_b)])
    # phase-1 lhsT blocks per matrix: (mat, i, c) at col ((mat*2+i)*4+c)*128
    pw = np.concatenate(
        [_pack_lhsT_blocks(Wm.astype(np.float32), 2, 4, np.float32)
         for Wm in mats], axis=1).astype(np.float16)
    pw = np.ascontiguousarray(pw)

    bias = np.zeros((128, 4 * NMAT), np.float32)
    for mi, bvec in enumerate(biases):
        bias[:, mi * 4:(mi + 1) * 4] = bvec.astype(np.float32).reshape(4, 128).T

    recs = [np.asarray(A0_w)] if z_i == 0 else [np.asarray(A0_w), np.asarray(A1_w)]
    # phase-2 lhsT block (m, c, k) at col ((m*4+c)*4+k)*128 = A_m[c128, k128].T
    aw_list = []
    for A in recs:
        Af = A.astype(np.float32)
        blocks = np.zeros((128, 16 * 128), np.float32)
        for c in range(4):
            for k in range(4):
                blocks[:, (c * 4 + k) * 128:(c * 4 + k + 1) * 128] = \
                    Af[c * 128:(c + 1) * 128, k * 128:(k + 1) * 128].T
        aw_list.append(blocks)
    aw = np.ascontiguousarray(np.concatenate(aw_list, axis=1).astype(np.float16))

    IDEN = np.ascontiguousarray(np.eye(128, dtype=np.float16))

    # ---- per-core inputs ----
    in_maps = []
    for cix in range(N_CORES):
        bc = cix * B_LOC
        xc = x_seq[bc:bc + B_LOC, :T]                       # [16, T, 256]
        xT = np.ascontiguousarray(
            xc.transpose(2, 1, 0).reshape(2, 128, T * B_LOC)).astype(np.float16)
        s0c = s0[bc:bc + B_LOC]                             # [16, 512]
        s0T = np.ascontiguousarray(
            s0c.T.reshape(4, 128, B_LOC).transpose(1, 0, 2).reshape(128, 64)
        ).astype(np.float16)
        in_maps.append({
            "xT": xT, "pw": pw, "bias": bias, "aw": aw, "s0T": s0T,
            "iden": IDEN,
        })

    res = run_bass_kernel_spmd(nc, in_maps, list(range(N_CORES)), trace=_trace)
    if _trace:
        kernel._last_res = res

    out = np.empty((B_FULL, T + 1, S_DIM), np.float32)
    for cix in range(N_CORES):
        bc = cix * B_LOC
        stg = np.asarray(res.results[cix]["stg"], dtype=np.float32)  # [T,128,64]
        out[bc:bc + B_LOC, 0] = s0[bc:bc + B_LOC]
        out[bc:bc + B_LOC, 1:] = (
            stg.reshape(T, 128, 4, B_LOC).transpose(3, 0, 2, 1)
            .reshape(B_LOC, T, S_DIM))
    return out


# revision 6
# speedup vs baseline: 1.5922x; 1.5922x over previous
"""Trainium2 Bass kernel for the BinaryMechanismSSM problem.

Full inputs in, full outputs out. Batch (128) sharded 8 ways (16 rows/core).

Per core:
  Phase 1: projections bx{0,1} = x @ B{0,1}^T + b (fp16 matmuls, fp32 psum),
           g = sigmoid(x @ G^T + b), 1-g = via DVE. Staged to contiguous
           per-(matrix, chunk) DRAM planes in fp16.
  Phase 2: T sequential steps. State held fp16 as sth[h][p, 16*cl+b] =
           state[b, 128*(2h+cl)+p] (two [128, 32] half tiles). Per step,
           per half h: one fp16 identity matmul injects bx into a
           [128, NREC*32] PSUM half-tile (start=True), 16 fp16 A-matmuls
           accumulate (contraction chunks k in {0,1} read sth[0], {2,3}
           read sth[1]); matmul emission is grouped so each psum half
           completes as soon as the late state half allows. Tail per half:
           ACT tanh -> DVE pair-add -> (0.5*w)*g -> + (1-g)*s_prev,
           emitting the new state half which gates the next step.
"""
import numpy as np

B_FULL = 128
T_FULL = 1024
I_DIM = 256
S_DIM = 512
N_CORES = 8
B_LOC = B_FULL // N_CORES  # 16

_cache = {}


def _build(alpha: float, z: int, T: int):
    import concourse.bass as bass
    from concourse import bacc
    import concourse.mybir as mybir
    from concourse.tile import TileContext

    dt = mybir.dt
    AF = mybir.ActivationFunctionType
    ALU = mybir.AluOpType

    TOK = T * B_LOC          # tokens per core
    NTT = TOK // 512         # phase-1 token tiles (32 timesteps each)
    NG = T // 16             # phase-2 step groups
    NMAT = 3 if z != 0 else 2
    NREC = 2 if z != 0 else 1
    HW = NREC * 32           # psum half width
    EQ = (z != 0 and abs(alpha - 0.5) < 1e-12)  # equal-coef fast path

    nc = bacc.Bacc("TRN2", target_bir_lowering=False, debug=False,
                   num_devices=N_CORES)

    xT_d = nc.declare_dram_parameter("xT", [2, 128, TOK], dt.float16, isOutput=False)
    pw_d = nc.declare_dram_parameter("pw", [128, NMAT * 2 * 4 * 128], dt.float16, isOutput=False)
    bias_d = nc.declare_dram_parameter("bias", [128, 4 * NMAT], dt.float32, isOutput=False)
    aw_d = nc.declare_dram_parameter("aw", [128, NREC * 16 * 128], dt.float16, isOutput=False)
    s0_d = nc.declare_dram_parameter("s0T", [128, 64], dt.float16, isOutput=False)
    iden_d = nc.declare_dram_parameter("iden", [128, 128], dt.float16, isOutput=False)
    stg_d = nc.declare_dram_parameter("stg", [T, 128, 64], dt.float16, isOutput=True)

    with TileContext(nc) as tc:
      with tc.tile_pool(name="dram", bufs=1, space="DRAM") as dpool:
        bxp = [[dpool.tile([128, TOK], dt.float16, tag=f"bxp{m}{c}",
                           name=f"bxp{m}{c}") for c in range(4)]
               for m in range(NREC)]
        hgp = [dpool.tile([128, TOK], dt.float16, tag=f"hgp{c}",
                          name=f"hgp{c}") for c in range(4)]
        gmp = [dpool.tile([128, TOK], dt.float16, tag=f"gmp{c}",
                          name=f"gmp{c}") for c in range(4)]

        # ---------------- Phase 1: projections ----------------
        with (
            tc.tile_pool(name="p1w", bufs=1) as p1w,
            tc.tile_pool(name="p1x", bufs=3) as p1x,
            tc.tile_pool(name="p1o", bufs=8) as p1o,
            tc.tile_pool(name="p1ps", bufs=6, space="PSUM") as p1ps,
        ):
            pw = p1w.tile([128, NMAT * 2 * 4 * 128], dt.float16)
            nc.sync.dma_start(pw[:], pw_d[:])
            bias = p1w.tile([128, 4 * NMAT], dt.float32)
            nc.sync.dma_start(bias[:], bias_d[:])

            for tt in range(NTT):
                xt = p1x.tile([128, 2 * 512], dt.float16, tag="xt")
                for i in range(2):
                    nc.sync.dma_start(xt[:, i * 512:(i + 1) * 512],
                                      xT_d[i, :, tt * 512:(tt + 1) * 512])
                for mat in range(NMAT):
                    for c in range(4):
                        ps = p1ps.tile([128, 512], dt.float32, tag="pps")
                        for i in range(2):
                            blk = ((mat * 2 + i) * 4 + c) * 128
                            nc.tensor.matmul(
                                ps[:], pw[:, blk:blk + 128],
                                xt[:, i * 512:(i + 1) * 512],
                                start=(i == 0), stop=(i == 1))
                        bj = bias[:, mat * 4 + c:mat * 4 + c + 1]
                        if mat < NREC:
                            o = p1o.tile([128, 512], dt.float16, tag="po")
                            nc.vector.tensor_scalar(
                                o[:], ps[:], bj, None, ALU.add)
                            nc.sync.dma_start(
                                bxp[mat][c][:, tt * 512:(tt + 1) * 512], o[:])
                        else:
                            sg = p1o.tile([128, 512], dt.float16, tag="psg")
                            nc.scalar.activation(sg[:], ps[:], AF.Sigmoid,
                                                 bias=bj, scale=1.0)
                            nc.sync.dma_start(
                                hgp[c][:, tt * 512:(tt + 1) * 512], sg[:])
                            gm = p1o.tile([128, 512], dt.float16, tag="pgm")
                            nc.vector.tensor_scalar(
                                gm[:], sg[:], -1.0, 1.0, ALU.mult, ALU.add)
                            nc.sync.dma_start(
                                gmp[c][:, tt * 512:(tt + 1) * 512], gm[:])

        # ---------------- Phase 2: recurrence ----------------
        with (
            tc.tile_pool(name="p2w", bufs=1) as p2w,
            tc.tile_pool(name="p2g", bufs=2) as p2g,
            tc.tile_pool(name="p2s", bufs=3) as p2s,
            tc.tile_pool(name="p2c", bufs=6) as p2c,
            tc.tile_pool(name="p2ps", bufs=4, space="PSUM") as p2ps,
        ):
            aw = p2w.tile([128, NREC * 16 * 128], dt.float16)
            nc.sync.dma_start(aw[:], aw_d[:])
            iden = p2w.tile([128, 128], dt.float16)
            nc.sync.dma_start(iden[:], iden_d[:])

            sth = []
            for h in range(2):
                s_init = p2s.tile([128, 32], dt.float16, tag=f"sth{h}")
                nc.sync.dma_start(s_init[:], s0_d[:, h * 32:(h + 1) * 32])
                sth.append(s_init)

            def ablk(m, c, k):
                return ((m * 4 + c) * 4 + k) * 128

            for g in range(NG):
                bxg = p2g.tile([128, NREC * 4 * 256], dt.float16, tag="bxg")
                for m in range(NREC):
                    for c in range(4):
                        nc.sync.dma_start(
                            bxg[:, (m * 4 + c) * 256:(m * 4 + c + 1) * 256],
                            bxp[m][c][:, g * 256:(g + 1) * 256])
                hgg = p2g.tile([128, 4 * 256], dt.float16, tag="hgg")
                gmg = p2g.tile([128, 4 * 256], dt.float16, tag="gmg")
                for c in range(4):
                    nc.sync.dma_start(hgg[:, c * 256:(c + 1) * 256],
                                      hgp[c][:, g * 256:(g + 1) * 256])
                    nc.sync.dma_start(gmg[:, c * 256:(c + 1) * 256],
                                      gmp[c][:, g * 256:(g + 1) * 256])
                bxg_v = bxg[:].rearrange("p (m c t b) -> p m c t b",
                                         m=NREC, c=4, t=16, b=16)
                hgg_v = hgg[:].rearrange("p (c t b) -> p c t b",
                                         c=4, t=16, b=16)
                gmg_v = gmg[:].rearrange("p (c t b) -> p c t b",
                                         c=4, t=16, b=16)

                for tt in range(16):
                    t = g * 16 + tt
                    psc = [p2ps.tile([128, HW], dt.float32, tag=f"psc{h}",
                                     name=f"psc{h}")
                           for h in range(2)]
                    for h in range(2):
                        nc.tensor.matmul(
                            psc[h][:], iden[:],
                            bxg_v[:, :, 2 * h:2 * h + 2, tt, :],
                            start=True, stop=False)

                    def amms(hreg, ks, stop_k):
                        for c in (2 * hreg, 2 * hreg + 1):
                            for m in range(NREC):
                                for k in ks:
                                    nc.tensor.matmul(
                                        psc[hreg][:, m * 32 + (c % 2) * 16:
                                                  m * 32 + (c % 2) * 16 + 16],
                                        aw[:, ablk(m, c, k):ablk(m, c, k) + 128],
                                        sth[k // 2][:, (k % 2) * 16:
                                                    (k % 2) * 16 + 16],
                                        start=False, stop=(k == stop_k))
                    amms(0, (0, 1), -1)   # h0 regions, early (sth0-gated)
                    amms(1, (0, 1), -1)   # h1 regions, early
                    amms(0, (2, 3), 3)    # h0 finishers (sth1-gated)
                    amms(1, (2, 3), 3)    # h1 finishers

                    new_sth = [None, None]
                    for h in (0, 1):
                        ft = p2c.tile([128, HW], dt.float16, tag=f"ft{h}")
                        nc.scalar.activation(ft[:], psc[h][:], AF.Tanh)
                        hgs = hgg_v[:, 2 * h:2 * h + 2, tt, :]
                        s_new = p2s.tile([128, 32], dt.float16, tag=f"sth{h}")
                        if NREC == 2 and EQ:
                            w = p2c.tile([128, 32], dt.float16, tag=f"w{h}")
                            nc.vector.tensor_tensor(
                                w[:], ft[:, 0:32], ft[:, 32:64], ALU.add)
                            q = p2c.tile([128, 32], dt.float16, tag=f"q{h}")
                            nc.vector.scalar_tensor_tensor(
                                q[:], w[:], 0.5, hgs, ALU.mult, ALU.mult)
                        elif NREC == 2:
                            mc = p2c.tile([128, 64], dt.float16, tag=f"mc{h}")
                            nc.vector.scalar_tensor_tensor(
                                mc[:, 0:32], ft[:, 0:32], 1.0 - alpha,
                                hgs, ALU.mult, ALU.mult)
                            nc.vector.scalar_tensor_tensor(
                                mc[:, 32:64], ft[:, 32:64], alpha,
                                hgs, ALU.mult, ALU.mult)
                            q = p2c.tile([128, 32], dt.float16, tag=f"q{h}")
                            nc.vector.tensor_tensor(
                                q[:], mc[:, 0:32], mc[:, 32:64], ALU.add)
                        else:
                            q = p2c.tile([128, 32], dt.float16, tag=f"q{h}")
                            nc.vector.scalar_tensor_tensor(
                                q[:], ft[:], 1.0, hgs, ALU.mult, ALU.mult)
                        m2h = p2c.tile([128, 32], dt.float16, tag=f"m2{h}")
                        nc.vector.tensor_tensor(
                            m2h[:], sth[h][:], gmg_v[:, 2 * h:2 * h + 2, tt, :],
                            ALU.mult)
                        nc.vector.tensor_tensor(s_new[:], q[:], m2h[:], ALU.add)
                        nc.sync.dma_start(stg_d[t][:, h * 32:(h + 1) * 32],
                                          s_new[:])
                        new_sth[h] = s_new
                    sth = new_sth

    nc.compile()
    return nc


def _pack_lhsT_blocks(Wm, kdim, mdim, dtype):
    """Wm: [mdim*128, kdim*128]; returns [128, kdim*mdim*128] with block
    (k, j) at cols (k*mdim+j)*128 equal to Wm[j-chunk, k-chunk].T."""
    out = np.zeros((128, kdim * mdim * 128), dtype=dtype)
    for k in range(kdim):
        for j in range(mdim):
            blk = Wm[j * 128:(j + 1) * 128, k * 128:(k + 1) * 128].T
            out[:, (k * mdim + j) * 128:(k * mdim + j + 1) * 128] = blk
    return np.ascontiguousarray(out)


def kernel(x_seq, s0, A0_w, B0_w, B0_b, A1_w, B1_w, B1_b, gate_w, gate_b,
           alpha, z, _T=None, _trace=False):
    from concourse.bass_utils import run_bass_kernel_spmd

    T = int(_T or T_FULL)
    alpha_f = float(np.asarray(alpha))
    z_i = int(np.asarray(z))

    key = (alpha_f, z_i, T)
    if key not in _cache:
        _cache[key] = _build(alpha_f, z_i, T)
    nc = _cache[key]

    NMAT = 3 if z_i != 0 else 2
    NREC = 2 if z_i != 0 else 1

    x_seq = np.asarray(x_seq, dtype=np.float32)
    s0 = np.asarray(s0, dtype=np.float32)

    # ---- shared (replicated) weight packing ----
    mats = ([np.asarray(B0_w), np.asarray(B1_w), np.asarray(gate_w)]
            if z_i != 0 else [np.asarray(B0_w), np.asarray(gate_w)])
    biases = ([np.asarray(B0_b), np.asarray(B1_b), np.asarray(gate_b)]
              if z_i != 0 else [np.asarray(B0_b), np.asarray(gate_b)])
    # phase-1 lhsT blocks per matrix: (mat, i, c) at col ((mat*2+i)*4+c)*128
    pw = np.concatenate(
        [_pack_lhsT_blocks(Wm.astype(np.float32), 2, 4, np.float32)
         for Wm in mats], axis=1).astype(np.float16)
    pw = np.ascontiguousarray(pw)

    bias = np.zeros((128, 4 * NMAT), np.float32)
    for mi, bvec in enumerate(biases):
        bias[:, mi * 4:(mi + 1) * 4] = bvec.astype(np.float32).reshape(4, 128).T

    recs = [np.asarray(A0_w)] if z_i == 0 else [np.asarray(A0_w), np.asarray(A1_w)]
    # phase-2 lhsT block (m, c, k) at col ((m*4+c)*4+k)*128 = A_m[c128, k128].T
    aw_list = []
    for A in recs:
        Af = A.astype(np.float32)
        blocks = np.zeros((128, 16 * 128), np.float32)
        for c in range(4):
            for k in range(4):
                blocks[:, (c * 4 + k) * 128:(c * 4 + k + 1) * 128] = \
                    Af[c * 128:(c + 1) * 128, k * 128:(k + 1) * 128].T
        aw_list.append(blocks)
    aw = np.ascontiguousarray(np.concatenate(aw_list, axis=1).astype(np.float16))

    IDEN = np.ascontiguousarray(np.eye(128, dtype=np.float16))

    # ---- per-core inputs ----
    in_maps = []
    for cix in range(N_CORES):
        bc = cix * B_LOC
        xc = x_seq[bc:bc + B_LOC, :T]                       # [16, T, 256]
        xT = np.ascontiguousarray(
            xc.transpose(2, 1, 0).reshape(2, 128, T * B_LOC)).astype(np.float16)
        s0c = s0[bc:bc + B_LOC]                             # [16, 512]
        s0T = np.ascontiguousarray(
            s0c.T.reshape(4, 128, B_LOC).transpose(1, 0, 2).reshape(128, 64)
        ).astype(np.float16)
        in_maps.append({
            "xT": xT, "pw": pw, "bias": bias, "aw": aw, "s0T": s0T,
            "iden": IDEN,
        })

    res = run_bass_kernel_spmd(nc, in_maps, list(range(N_CORES)), trace=_trace)
    if _trace:
        kernel._last_res = res

    out = np.empty((B_FULL, T + 1, S_DIM), np.float32)
    for cix in range(N_CORES):
        bc = cix * B_LOC
        stg = np.asarray(res.results[cix]["stg"], dtype=np.float32)  # [T,128,64]
        out[bc:bc + B_LOC, 0] = s0[bc:bc + B_LOC]
        out[bc:bc + B_LOC, 1:] = (
            stg.reshape(T, 128, 4, B_LOC).transpose(3, 0, 2, 1)
            .reshape(B_LOC, T, S_DIM))
    return out
